# revision 1
# baseline (speedup 1.0000x reference)
"""Multi-head attention (QKV proj + SDPA + output proj) on 8 Trainium2 cores.

Sharding: tensor-parallel over heads. 16 heads / 8 cores = 2 heads per core.
Each core computes q/k/v for its 2 heads, SDPA, and a partial output
projection against its 128-column slice of proj_w. The host sums the 8
partial projections (the all-reduce step done host-side, since this kernel
returns full outputs anyway).

Device-side layouts (per core, T = transposed so the contraction dim is on
SBUF partitions):
  xT   [1024, 4096]  x transposed (host-prepped), bf16
  wqk  [1024, 256]   [wq_c.T | wk_c.T] for the core's 2 heads, bf16
  wv   [1024, 128]   wv_c.T, bf16
  pw   [128, 1024]   proj_w[:, core cols].T, bf16
  bqk  [128, 2]      q/k biases (per-partition in qT/kT layout), f32
  out: partialT [1024, 4096] f32 = (attn_out @ proj_w_c.T).T, no biases.

The v bias and proj bias are linear post-terms: attn weights sum to 1, so
v_bias contributes qkv_b[2048:] @ proj_w.T to every row — added on host.

Softmax skips the max-subtraction: scores have std ~1 (scale=1/8, d=64,
unit-variance q/k), so exp() stays in fp32 range with huge margin.
"""

import numpy as np
import ml_dtypes

N_CORES = 8
SEQ = 4096
DMODEL = 1024
NHEADS = 16
DHEAD = 64
H_PER_CORE = NHEADS // N_CORES  # 2
CBLK = DMODEL // N_CORES  # 128 head-dim columns per core

IT = 512  # i (query) tile width
NI = SEQ // IT  # 8
JT = 128  # j (key) tile = psum partition dim
NJ = SEQ // JT  # 32
NCT = DMODEL // 128  # 8 contraction tiles for the projections
SCALE = DHEAD ** -0.5

# j-tiles per exp chunk: 2 fp32 psum banks per chunk; shared-tag bufs=3 (6
# banks) + 2 av banks = 8. Depth 3 keeps PE fed while ACT exps, so the HAM
# clock gate stays at 8/8.
CHUNKS = [3] * 10 + [2]  # sums to NJ=32

_CACHE = {}


def _build_nc():
    import concourse.tile as tile
    from concourse import bacc, mybir

    bf16 = mybir.dt.bfloat16
    f16 = mybir.dt.float16
    f32 = mybir.dt.float32
    Exp = mybir.ActivationFunctionType.Exp

    nc = bacc.Bacc(
        "TRN2",
        target_bir_lowering=False,
        debug=False,
        enable_asserts=True,
        num_devices=N_CORES,
    )

    xT = nc.dram_tensor("xT", [DMODEL, SEQ], bf16, kind="ExternalInput").ap()
    wqk = nc.dram_tensor("wqk", [DMODEL, 256], bf16, kind="ExternalInput").ap()
    wv = nc.dram_tensor("wv", [DMODEL, CBLK], bf16, kind="ExternalInput").ap()
    pw = nc.dram_tensor("pw", [CBLK, DMODEL], bf16, kind="ExternalInput").ap()
    bqk = nc.dram_tensor("bqk", [128, 2], f32, kind="ExternalInput").ap()
    partialT = nc.dram_tensor(
        "partialT", [DMODEL, SEQ], f16, kind="ExternalOutput"
    ).ap()

    with tile.TileContext(nc) as tc:
        with (
            tc.tile_pool(name="weights", bufs=1) as wpool,
            tc.tile_pool(name="xtiles", bufs=NCT) as xpool,
            tc.tile_pool(name="qk", bufs=1) as qkpool,
            tc.tile_pool(name="vaug", bufs=NJ) as vpool,
            tc.tile_pool(name="exps", bufs=2) as epool,
            tc.tile_pool(name="attn", bufs=1) as apool,
            tc.tile_pool(name="norm", bufs=4) as npool,
            tc.tile_pool(name="stage", bufs=4) as stpool,
        ):
            # ---- load weights + x (wqk_c0 + x_c0 first so the first qk
            # matmuls start as soon as possible; wv/pw are needed later) ----
            wqk_t = []
            xt = []
            for c in range(NCT):
                wqk_c = wpool.tile([128, 256], bf16, name=f"wqk_c{c}")
                nc.sync.dma_start(wqk_c[:], wqk[c * 128 : (c + 1) * 128, :])
                wqk_t.append(wqk_c)
                x_c = xpool.tile([128, SEQ], bf16, name=f"x_c{c}", tag="xc")
                nc.sync.dma_start(x_c[:], xT[c * 128 : (c + 1) * 128, :])
                xt.append(x_c)
            bqk_t = wpool.tile([128, 2], f32)
            nc.sync.dma_start(bqk_t[:], bqk[:])
            wv_t = []
            for c in range(NCT):
                wv_c = wpool.tile([128, CBLK], bf16, name=f"wv_c{c}")
                nc.sync.dma_start(wv_c[:], wv[c * 128 : (c + 1) * 128, :])
                wv_t.append(wv_c)
            pw_t = wpool.tile([128, DMODEL], bf16)
            nc.sync.dma_start(pw_t[:], pw[:])

            # ---- QKV projections ----
            # qT/kT: [2*DHEAD=128, SEQ], stationary = w slices, moving = xT
            qT = qkpool.tile([128, SEQ], bf16)
            kT = qkpool.tile([128, SEQ], bf16)
            psqk = tc.tile_pool(name="psqk", bufs=1, space="PSUM")
            pspool = psqk.__enter__()
            for f, dest in ((0, qT), (1, kT)):
                ps = []
                for i in range(NI):
                    p = pspool.tile(
                        [128, IT], f32, name=f"qkps{f}_{i}", tag="qkps", bufs=8
                    )
                    ps.append(p)
                for c in range(NCT):
                    lhsT = wqk_t[c][:, f * 128 : (f + 1) * 128]
                    for i in range(NI):
                        nc.tensor.matmul(
                            ps[i][:],
                            lhsT,
                            xt[c][:, i * IT : (i + 1) * IT],
                            start=(c == 0),
                            stop=(c == NCT - 1),
                        )
                for i in range(NI):
                    if i % 2 == 0:
                        nc.vector.tensor_scalar_add(
                            dest[:, i * IT : (i + 1) * IT],
                            ps[i][:],
                            bqk_t[:, f : f + 1],
                        )
                    else:
                        nc.scalar.activation(
                            dest[:, i * IT : (i + 1) * IT],
                            ps[i][:],
                            mybir.ActivationFunctionType.Identity,
                            bias=bqk_t[:, f : f + 1],
                        )

            # v in natural layout [j, d] (+ ones column per head for the
            # softmax denominator): stationary = xT block, moving = wv.
            psqk.__exit__(None, None, None)
            psv = tc.tile_pool(name="psv", bufs=1, space="PSUM")
            pspool = psv.__enter__()
            vaug = []
            for j in range(NJ):
                vp = pspool.tile([128, CBLK], f32, name=f"vps{j}", tag="vps", bufs=2)
                for c in range(NCT):
                    nc.tensor.matmul(
                        vp[:],
                        xt[c][:, j * JT : (j + 1) * JT],
                        wv_t[c][:],
                        start=(c == 0),
                        stop=(c == NCT - 1),
                    )
                va = vpool.tile([128, 130], bf16, name=f"vaug{j}", tag="vaug")
                nc.vector.tensor_copy(va[:, 0:64], vp[:, 0:64])
                nc.vector.tensor_copy(va[:, 65:129], vp[:, 64:128])
                nc.vector.memset(va[:, 64:65], 1.0)
                nc.vector.memset(va[:, 129:130], 1.0)
                vaug.append(va)

            # ---- attention ----
            # scoresT[j, i] = k . q per head; exp on ACT (scale folded in);
            # av = v_aug.T @ expT accumulated over j; row 64 = denominator.
            psv.__exit__(None, None, None)
            psattn = tc.tile_pool(name="psattn", bufs=1, space="PSUM")
            pspool = psattn.__enter__()
            attn_outT = apool.tile([128, SEQ], bf16)
            for i in range(NI):
                av = [
                    pspool.tile([128, IT], f32, name=f"av0_{i}", tag="av0"),
                    pspool.tile([128, IT], f32, name=f"av1_{i}", tag="av1"),
                ]
                jbase = 0
                for ci, csz in enumerate(CHUNKS):
                    sc = [
                        pspool.tile(
                            [128, 3 * IT], f32, name=f"sc0_{i}_{ci}", tag="sc",
                            bufs=2,
                        ),
                        pspool.tile(
                            [128, 3 * IT], f32, name=f"sc1_{i}_{ci}", tag="sc",
                            bufs=2,
                        ),
                    ]
                    for t in range(csz):
                        j = jbase + t
                        for h in range(2):
                            nc.tensor.matmul(
                                sc[h][:, t * IT : (t + 1) * IT],
                                kT[h * 64 : (h + 1) * 64, j * JT : (j + 1) * JT],
                                qT[h * 64 : (h + 1) * 64, i * IT : (i + 1) * IT],
                                start=True,
                                stop=True,
                                tile_position=(h * 64, 0),
                            )
                    et = []
                    for h in range(2):
                        e = epool.tile(
                            [128, 3 * IT], bf16, name=f"e{h}_{i}_{ci}",
                            tag=f"e{h}", bufs=2,
                        )
                        nc.scalar.activation(
                            e[:, 0 : csz * IT],
                            sc[h][:, 0 : csz * IT],
                            Exp,
                            scale=SCALE,
                        )
                        et.append(e)
                    for t in range(csz):
                        j = jbase + t
                        for h in range(2):
                            nc.tensor.matmul(
                                av[h][0:65, :],
                                vaug[j][:, h * 65 : h * 65 + 65],
                                et[h][:, t * IT : (t + 1) * IT],
                                start=(j == 0),
                                stop=(j == NJ - 1),
                            )
                    jbase += csz

                # Copy av out of PSUM immediately (frees the bank for the
                # next i-tile); normalize runs off the critical path.
                for h in range(2):
                    avs = npool.tile(
                        [128, IT], f32, name=f"avs{h}_{i}", tag="avs", bufs=4
                    )
                    nc.vector.tensor_copy(avs[:65, :], av[h][0:65, :])
                    rd = npool.tile([1, IT], f32, name=f"rd{h}_{i}", tag="rd")
                    nc.vector.reciprocal(rd[:], avs[64:65, :])
                    rb = npool.tile([64, IT], f32, name=f"rb{h}_{i}", tag="rb")
                    nc.gpsimd.partition_broadcast(rb[:], rd[:], channels=64)
                    nc.vector.tensor_mul(
                        attn_outT[h * 64 : (h + 1) * 64, i * IT : (i + 1) * IT],
                        avs[0:64, :],
                        rb[:],
                    )

            psattn.__exit__(None, None, None)
            psproj = tc.tile_pool(name="psproj", bufs=1, space="PSUM")
            pspool = psproj.__enter__()
            # ---- output projection (partial, this core's 128 hd columns) ----
            # psum drains alternate DVE/ACT (both idle here) so the drain
            # rate keeps up with the matmuls.
            for cc in range(NCT):
                lhsT = pw_t[:, cc * 128 : (cc + 1) * 128]
                for i in range(NI):
                    pp = pspool.tile(
                        [128, IT], f32, name=f"pp{cc}_{i}", tag="pp", bufs=4
                    )
                    nc.tensor.matmul(
                        pp[:],
                        lhsT,
                        attn_outT[:, i * IT : (i + 1) * IT],
                        start=True,
                        stop=True,
                    )
                    st = stpool.tile(
                        [128, IT], f16, name=f"st{cc}_{i}", tag="st", bufs=6
                    )
                    if i % 2 == 0:
                        nc.vector.tensor_copy(st[:], pp[:])
                    else:
                        nc.scalar.copy(st[:], pp[:])
                    nc.sync.dma_start(
                        partialT[
                            cc * 128 : (cc + 1) * 128, i * IT : (i + 1) * IT
                        ],
                        st[:],
                    )
            psproj.__exit__(None, None, None)

    nc.compile()
    return nc


def _get_nc():
    if "nc" not in _CACHE:
        _CACHE["nc"] = _build_nc()
    return _CACHE["nc"]


def kernel(x, qkv_w, qkv_b, proj_w, proj_b):
    from concourse.bass_utils import run_bass_kernel_spmd

    nc = _get_nc()

    bf16 = ml_dtypes.bfloat16
    x2d = np.ascontiguousarray(x.reshape(SEQ, DMODEL).T).astype(bf16)  # [1024, 4096]

    in_maps = []
    for c in range(N_CORES):
        lo, hi = c * CBLK, (c + 1) * CBLK
        wq_c = qkv_w[lo:hi, :]  # [128, 1024]
        wk_c = qkv_w[DMODEL + lo : DMODEL + hi, :]
        wv_c = qkv_w[2 * DMODEL + lo : 2 * DMODEL + hi, :]
        in_maps.append(
            {
                "xT": x2d,
                "wqk": np.ascontiguousarray(
                    np.concatenate([wq_c.T, wk_c.T], axis=1)
                ).astype(bf16),
                "wv": np.ascontiguousarray(wv_c.T).astype(bf16),
                "pw": np.ascontiguousarray(proj_w[:, lo:hi].T).astype(bf16),
                "bqk": np.ascontiguousarray(
                    np.stack(
                        [qkv_b[lo:hi], qkv_b[DMODEL + lo : DMODEL + hi]], axis=1
                    )
                ).astype(np.float32),
            }
        )

    res = run_bass_kernel_spmd(nc, in_maps, core_ids=list(range(N_CORES)))

    acc = np.zeros((DMODEL, SEQ), dtype=np.float32)
    for c in range(N_CORES):
        acc += res.results[c]["partialT"].astype(np.float32)

    # host-side linear bias terms: proj bias + v-bias routed through proj
    bias = qkv_b[2 * DMODEL :].astype(np.float32) @ proj_w.T.astype(
        np.float32
    ) + proj_b.astype(np.float32)
    out = acc.T + bias[None, :]
    return out.reshape(1, SEQ, DMODEL).astype(np.float32)

